# revision 12
# baseline (speedup 1.0000x reference)
"""KNN IDW flow interpolation (AccFlow) v3 — 8 TRN2 cores.

v2's segment-max candidate machinery + a refine stage that REPRODUCES the
reference's fp32 arithmetic. The fp32 reference computes
    d2 = fl(fl(q2 + r2) - fl(2 * dot_fma(q, r)))
where dot_fma is a left-to-right fp32 FMA chain (what BLAS/XLA emit for
K=3). d2 for near-duplicate pairs is dominated by that rounding noise, so
an EXACT kernel mismatches the reference (rel err 0.039); reproducing the
rounding passes. DVE/ACT have no FMA, so each chain step
fl(a*b + acc) is emulated exactly with a Dekker product (12-bit Veltkamp
splits precomputed on host) + TwoSum compensation — all fp32-exact.

Per 128-query tile:
  1. PE: fused 12-row hi/lo fp16 matmul -> coarse scores (error ~1e-4,
     selection only).
  2. DVE: segment max from PSUM [128, 2048 segs of 8]; top-8 segments
     provably contain the true top-8 refs.
  3. max8 + max_index -> 8 segment ids; indirect-DMA gather of packed
     segment rows (coords, splits, r2, flow).
  4. ACT+DVE: emulated-fp32 d2 per candidate, top-8 of 64 by threshold,
     w = 1/(d2+eps), normalized weighted flow sum.
"""

import numpy as np

N = 32768
M = 16384
K = 8
EPS = 1e-8
NCORES = 8
NSHARD = N // NCORES  # 4096
P = 128
CHUNK = 512
SEG = 8
NSEGS = M // SEG      # 2048
ROWS = 15             # fused hi/lo aug rows (5-row aug: s = -d2)
CAND = 64             # 8 segments x 8 refs
RCW = 16              # floats per ref row in refcat

_CACHE = {}


def _build_nc(nshard):
    import concourse.bass as bass
    import concourse.bacc as bacc
    import concourse.mybir as mybir
    from concourse.tile import TileContext

    f32 = mybir.dt.float32
    f16 = mybir.dt.float16
    u32 = mybir.dt.uint32
    ACT = mybir.ActivationFunctionType
    ntiles = nshard // P

    nc = bacc.Bacc()
    q12 = nc.dram_tensor("q12", [ROWS, nshard], f16, kind="ExternalInput")
    r12 = nc.dram_tensor("r12", [ROWS, M], f16, kind="ExternalInput")
    # qt rows: [q0,q1,q2,q2sum, q0h,q0l,q1h,q1l,q2h,q2l, 0,0]
    qt = nc.dram_tensor("qt", [nshard, 12], f32, kind="ExternalInput")
    # refcat ref layout: [r0,r1,r2,r2sum, fx,fy,fz,0, r0h,r0l,r1h,r1l,r2h,r2l,0,0]
    refcat = nc.dram_tensor("refcat", [NSEGS, SEG * RCW], f32,
                            kind="ExternalInput")
    out = nc.dram_tensor("out", [nshard, 3], f32, kind="ExternalOutput")

    GRP = 4 * CHUNK   # psum group = 4 banks = 2048 scores = 256 segments
    NGRP = M // GRP   # 8

    with TileContext(nc) as tc:
        with (
            tc.tile_pool(name="const", bufs=1) as cpool,
            tc.tile_pool(name="psum", bufs=2, space="PSUM") as psum_pool,
            tc.tile_pool(name="qtile", bufs=3) as qtile_pool,
            tc.tile_pool(name="seg", bufs=2) as seg_pool,
            tc.tile_pool(name="tree", bufs=2) as tree_pool,
            tc.tile_pool(name="cand", bufs=3) as cand_pool,
            tc.tile_pool(name="fma", bufs=3) as fma_pool,
            tc.tile_pool(name="small", bufs=3) as small,
        ):
            r_sb = cpool.tile([ROWS, M], f16)
            nc.sync.dma_start(out=r_sb[:], in_=r12[:])

            # q-tile DMAs are prefetched one iteration ahead so the Sync
            # queue's out-DMA of tile t (which waits on tile t's refine)
            # cannot head-of-line-block the inputs of tile t+1.
            def qload(ti):
                qs = qtile_pool.tile([ROWS, P], f16, tag="q", name="qs")
                nc.sync.dma_start(out=qs[:], in_=q12[:, bass.ts(ti, P)])
                qts = qtile_pool.tile([P, 12], f32, tag="qt", name="qts")
                nc.sync.dma_start(out=qts[:], in_=qt[bass.ts(ti, P), :])
                return qs, qts

            nxt = qload(0)
            pending = None
            for t in range(ntiles):
                qsl = bass.ts(t, P)
                q_sb, qt_sb = nxt
                if t + 1 < ntiles:
                    nxt = qload(t + 1)

                # ACT converts each PSUM group to fp16 in one wide tile; DVE
                # then reduces the 8 interleaved groups with a 3-op pairwise
                # max tree (fp16 2x mode). Segment j = refs {j + 2048*g}.
                w16 = seg_pool.tile([P, M], f16, tag="w16")
                for g in range(NGRP):
                    ps = psum_pool.tile([P, GRP], f32, tag="ps")
                    for b in range(4):
                        c = g * 4 + b
                        nc.tensor.matmul(ps[:, bass.ts(b, CHUNK)], q_sb[:],
                                         r_sb[:, bass.ts(c, CHUNK)],
                                         start=True, stop=True)
                    nc.scalar.activation(w16[:, bass.ts(g, GRP)], ps[:],
                                         mybir.ActivationFunctionType.Copy)
                t1 = tree_pool.tile([P, M // 2], f16, tag="t1")
                nc.vector.tensor_tensor(out=t1[:], in0=w16[:, :M // 2],
                                        in1=w16[:, M // 2:],
                                        op=mybir.AluOpType.max)
                t2 = tree_pool.tile([P, M // 4], f16, tag="t2")
                nc.vector.tensor_tensor(out=t2[:], in0=t1[:, :M // 4],
                                        in1=t1[:, M // 4:],
                                        op=mybir.AluOpType.max)
                segmax = tree_pool.tile([P, NSEGS], f16, tag="segmax")
                nc.vector.tensor_tensor(out=segmax[:], in0=t2[:, :NSEGS],
                                        in1=t2[:, NSEGS:],
                                        op=mybir.AluOpType.max)

                top8 = small.tile([P, K], f16, tag="top8")
                sidx = small.tile([P, K], u32, tag="sidx")
                nc.vector.max(out=top8[:], in_=segmax[:])
                nc.vector.max_index(out=sidx[:], in_max=top8[:],
                                    in_values=segmax[:])

                cand = cand_pool.tile([P, K * SEG * RCW], f32, tag="cand")
                for k in range(K):
                    nc.gpsimd.indirect_dma_start(
                        out=cand[:, k * SEG * RCW:(k + 1) * SEG * RCW],
                        out_offset=None,
                        in_=refcat[:],
                        in_offset=bass.IndirectOffsetOnAxis(
                            ap=sidx[:, k:k + 1], axis=0),
                    )
                def refine(cand, qt_sb, qsl):
                    cv = cand[:].rearrange("p (n w) -> p n w", w=RCW)

                    def ftile(tag):
                        ft = fma_pool.tile([P, CAND], f32, tag=tag, name=tag)
                        return ft

                    def f2tile(tag):
                        ft = fma_pool.tile([P, 2 * CAND], f32, tag=tag,
                                           name=tag)
                        return ft

                    # g = fl(q0*r0); then two emulated-FMA steps. Dekker
                    # exact products for c=1,2 batched in [P, 2, CAND] DVE
                    # ops (q broadcast over candidates); TwoSum sequential
                    # (acc dependency). All on DVE: any ACT op here would
                    # head-of-line-block the next tile's PSUM copies on the
                    # Scalar queue.
                    acc = ftile("acc")
                    nc.vector.tensor_scalar_mul(acc[:], cv[:, :, 0],
                                                qt_sb[:, 0:1])
                    cw = cand[:].rearrange("p (n w) -> p w n", w=RCW)
                    mul = mybir.AluOpType.mult
                    p2 = f2tile("p2")
                    w12 = f2tile("w12")
                    w22 = f2tile("w22")
                    w32 = f2tile("w32")
                    w42 = f2tile("w42")
                    d12 = f2tile("d12")
                    e2 = f2tile("e2")

                    def bprod(out, bcols, acols):
                        nc.vector.tensor_tensor(
                            out=out[:].rearrange("p (c n) -> p c n", c=2),
                            in0=bcols,
                            in1=acols.to_broadcast([P, 2, CAND]),
                            op=mul)

                    bprod(p2, cw[:, 1:3, :], qt_sb[:, 1:3])
                    bprod(w12, cw[:, 10:14:2, :], qt_sb[:, 6:10:2])
                    bprod(w22, cw[:, 11:15:2, :], qt_sb[:, 6:10:2])
                    bprod(w32, cw[:, 10:14:2, :], qt_sb[:, 7:11:2])
                    bprod(w42, cw[:, 11:15:2, :], qt_sb[:, 7:11:2])
                    nc.vector.tensor_sub(d12[:], p2[:], w12[:])
                    nc.vector.tensor_sub(d12[:], d12[:], w22[:])
                    nc.vector.tensor_sub(d12[:], d12[:], w32[:])
                    nc.vector.tensor_sub(e2[:], w42[:], d12[:])  # ab - p
                    for c in (1, 2):
                        csl = slice((c - 1) * CAND, c * CAND)
                        p = p2[:, csl]
                        e = e2[:, csl]
                        s = ftile("s")
                        nc.vector.tensor_add(s[:], p, acc[:])
                        z = ftile("z")
                        nc.vector.tensor_sub(z[:], s[:], p)
                        zz = ftile("zz")
                        nc.vector.tensor_sub(zz[:], s[:], z[:])
                        t1_ = ftile("t1_")
                        nc.vector.tensor_sub(t1_[:], acc[:], z[:])
                        t2_ = ftile("t2_")
                        nc.vector.tensor_sub(t2_[:], p, zz[:])
                        nc.vector.tensor_add(t1_[:], t1_[:], t2_[:])
                        nc.vector.tensor_add(t1_[:], t1_[:], e)
                        acc2 = ftile("acc")
                        nc.vector.tensor_add(acc2[:], s[:], t1_[:])
                        acc = acc2

                    # d2 = relu(fl(fl(q2+r2) - fl(2*g))); weights use d2
                    # directly (the reference's sqrt->square double-rounding
                    # changes weights only at the 1e-7 level).
                    s1 = ftile("s1")
                    nc.vector.tensor_scalar_add(s1[:], cv[:, :, 3],
                                                qt_sb[:, 3:4])
                    # d2neg = fl(fl(2g) - s1) = -d2 exactly (IEEE negation);
                    # s2 = min(d2neg, 0) = -max(d2, 0) in one op.
                    d2n = ftile("d2n")
                    nc.vector.scalar_tensor_tensor(
                        d2n[:], acc[:], 2.0, s1[:],
                        op0=mybir.AluOpType.mult,
                        op1=mybir.AluOpType.subtract)
                    s2 = ftile("s2")
                    nc.vector.tensor_scalar_min(s2[:], d2n[:], 0.0)

                    t8 = small.tile([P, K], f32, tag="t8")
                    nc.vector.max(out=t8[:], in_=s2[:])
                    r1 = ftile("r1")
                    nc.vector.tensor_scalar(r1[:], s2[:], -1.0, EPS,
                                            op0=mybir.AluOpType.mult,
                                            op1=mybir.AluOpType.add)
                    nc.vector.reciprocal(r1[:], r1[:])
                    w = ftile("w")
                    wsum = small.tile([P, 1], f32, tag="wsum")
                    nc.vector.scalar_tensor_tensor(
                        w[:], s2[:], t8[:, K - 1:K], r1[:],
                        op0=mybir.AluOpType.is_ge, op1=mybir.AluOpType.mult,
                        accum_out=wsum[:])
                    winv = small.tile([P, 1], f32, tag="winv")
                    nc.vector.reciprocal(winv[:], wsum[:])

                    wf = small.tile([P, CAND * 3], f32, tag="wf")
                    nc.vector.tensor_tensor(
                        out=wf[:].rearrange("p (n c) -> p n c", c=3),
                        in0=cv[:, :, 4:7],
                        in1=w[:].to_broadcast([P, CAND, 3]),
                        op=mul)
                    o3 = small.tile([P, 3], f32, tag="o3")
                    nc.vector.reduce_sum(
                        o3[:], wf[:].rearrange("p (n c) -> p c n", c=3),
                        axis=mybir.AxisListType.X)
                    res = small.tile([P, 3], f32, tag="res")
                    nc.vector.tensor_scalar_mul(res[:], o3[:], winv[:])
                    nc.sync.dma_start(out=out[qsl, :], in_=res[:])

                # refine runs one tile behind selection so the DVE queue
                # never stalls on the current tile's gathers.
                if pending is not None:
                    refine(*pending)
                pending = (cand, qt_sb, qsl)
            refine(*pending)

    nc.compile()
    return nc


def _vsplit(x):
    """Veltkamp split of fp32 array into 12-bit hi + lo halves."""
    x = np.asarray(x, np.float32)
    c = np.float32(4097.0)  # 2^12 + 1
    t = np.float32(c * x)
    hi = np.float32(t - np.float32(t - x))
    lo = np.float32(x - hi)
    return hi, lo


def _host_prep(query_points, ref_points, ref_flow):
    q = np.asarray(query_points, dtype=np.float32)
    r = np.asarray(ref_points, dtype=np.float32)
    f = np.ascontiguousarray(np.asarray(ref_flow, dtype=np.float32))
    r2 = (r * r).sum(1, dtype=np.float32)
    q2 = (q * q).sum(1, dtype=np.float32)
    onesN = np.ones(q.shape[0], np.float32)
    onesM = np.ones(r.shape[0], np.float32)
    A = np.stack([q[:, 0], q[:, 1], q[:, 2], q2, onesN], 0).astype(np.float32)
    B = np.stack([2 * r[:, 0], 2 * r[:, 1], 2 * r[:, 2], -onesM, -r2], 0)
    B = np.ascontiguousarray(B).astype(np.float32)
    Ah = A.astype(np.float16)
    Al = (A - Ah.astype(np.float32)).astype(np.float16)
    Bh = B.astype(np.float16)
    Bl = (B - Bh.astype(np.float32)).astype(np.float16)
    q12 = np.ascontiguousarray(np.concatenate([Ah, Ah, Al], 0))
    r12 = np.ascontiguousarray(np.concatenate([Bh, Bl, Bh], 0))

    qh, ql = _vsplit(q)
    qt = np.zeros((q.shape[0], 12), np.float32)
    qt[:, 0:3] = q
    qt[:, 3] = q2
    qt[:, 4:10:2] = qh
    qt[:, 5:10:2] = ql

    rh, rl = _vsplit(r)
    refcat = np.zeros((M, RCW), np.float32)
    refcat[:, 0:3] = r
    refcat[:, 3] = r2
    refcat[:, 4:7] = f
    refcat[:, 8:14:2] = rh
    refcat[:, 9:14:2] = rl
    # interleaved segments: segment j = refs {j + NSEGS*g, g=0..7}
    refcat = refcat.reshape(SEG, NSEGS, RCW).transpose(1, 0, 2)
    refcat = np.ascontiguousarray(refcat.reshape(NSEGS, SEG * RCW))
    return q12, r12, np.ascontiguousarray(qt), refcat


def _run(inputs, trace=False):
    from concourse.bass_utils import run_bass_kernel_spmd

    q12, r12, qt, refcat = _host_prep(
        inputs["query_points"], inputs["ref_points"], inputs["ref_flow"])
    if NSHARD not in _CACHE:
        _CACHE[NSHARD] = _build_nc(NSHARD)
    nc = _CACHE[NSHARD]

    in_maps = []
    for i in range(NCORES):
        sl = slice(i * NSHARD, (i + 1) * NSHARD)
        in_maps.append({
            "q12": np.ascontiguousarray(q12[:, sl]),
            "r12": r12,
            "qt": np.ascontiguousarray(qt[sl]),
            "refcat": refcat,
        })

    res = run_bass_kernel_spmd(nc, in_maps, core_ids=list(range(NCORES)),
                               trace=trace)
    outs = [res.results[i]["out"] for i in range(NCORES)]
    full = np.concatenate(outs, axis=0)
    return full, res.exec_time_ns


def kernel(query_points, ref_points, ref_flow):
    out, _ = _run({"query_points": query_points, "ref_points": ref_points,
                   "ref_flow": ref_flow})
    return out
